# revision 3
# baseline (speedup 1.0000x reference)
"""Sharded brute-force kNN (cosine-sim top-k) on 8 Trainium2 NeuronCores.

Strategy (passage-row-wise sharding):
  - Each core gets a 32768-passage shard (of 262144) plus the full 2048
    queries, both pre-transposed host-side to K-major layout.
  - Device: S = Q @ P_shard.T computed as 6 fp32 k-tile matmuls per
    [128-query, 512-passage] PSUM bank; the DVE max8/max_index pair then
    extracts the top-8 values+indices of each 512-passage chunk directly
    from PSUM (PE and DVE overlap; sims never round-trip through SBUF).
  - Per-chunk top-8 over 512 global chunks is a superset of each row's
    global top-100 unless one 512-wide chunk holds >=9 of that row's
    top-100 (P ~ 1e-6 per full run for iid inputs).
  - Host: merge the 8x512 candidates per row and take an exact stable
    top-k (ties broken by lowest passage index, matching jax.lax.top_k).
"""
import numpy as np

import concourse.bacc as bacc
import concourse.tile as tile
from concourse import mybir
from concourse.bass_utils import run_bass_kernel_spmd

P = 128
Q = 2048              # queries (replicated on all cores)
D = 768               # embedding dim = 6 k-tiles of 128
NCORES = 8
NTOTAL = 262144       # total passages
NSH = NTOTAL // NCORES  # 32768 passages per core
CHUNK = 512           # passages per PSUM bank
NCHUNK = NSH // CHUNK  # 64
NQB = Q // P          # 16 query blocks
KT = D // P           # 6 k-tiles

TRACE = False         # set True (e.g. from test.py) to capture an NTFF profile
LAST_PERF = None      # BassKernelResults of the last run when TRACE was set

MODE = "f32r"         # "f32r": fast TF32-class matmul + exact host rescore
                      # "f32": native fp32 matmul (4 cycles/row), no rescore needed
RESCORE = 128         # candidates per row rescored exactly on host (f32r mode)
SBUF_MAX8 = True      # bounce sims PSUM->SBUF on the idle ACT engine and run
                      # the DVE max8/find_index8 scans from SBUF (faster reads)

_NC_CACHE = {}


def _build(mode):
    mm_dt = mybir.dt.float32 if mode == "f32" else mybir.dt.float32r
    nc = bacc.Bacc("TRN2", target_bir_lowering=False)
    qt = nc.dram_tensor("qt", [D, Q], mybir.dt.float32, kind="ExternalInput")
    pt = nc.dram_tensor("pt", [D, NSH], mybir.dt.float32, kind="ExternalInput")
    vals = nc.dram_tensor("vals", [Q, NCHUNK * 8], mybir.dt.float32, kind="ExternalOutput")
    idx = nc.dram_tensor("idx", [Q, NCHUNK * 8], mybir.dt.uint32, kind="ExternalOutput")

    qt_ap = qt.ap().rearrange("(s p) q -> p s q", p=P)   # [128, 6, 2048]
    pt_ap = pt.ap().rearrange("(s p) n -> p s n", p=P)   # [128, 6, 32768]
    if mm_dt != mybir.dt.float32:
        qt_ap = qt_ap.bitcast(mm_dt)
        pt_ap = pt_ap.bitcast(mm_dt)

    with tile.TileContext(nc) as tc:
        with (
            tc.tile_pool(name="qpool", bufs=1) as qpool,
            tc.tile_pool(name="ppool", bufs=3) as ppool,
            tc.tile_pool(name="opool", bufs=1) as opool,
            tc.tile_pool(name="cpool", bufs=4) as cpool,
            tc.tile_pool(name="pspool", bufs=8, space="PSUM") as pspool,
        ):
            qt_t = qpool.tile([P, KT, Q], mm_dt, name="qt_t")
            nc.sync.dma_start(qt_t[:], qt_ap)

            ovals = [opool.tile([P, NCHUNK * 8], mybir.dt.float32, tag=f"ov{b}", name=f"ov{b}")
                     for b in range(NQB)]
            oidx = [opool.tile([P, NCHUNK * 8], mybir.dt.uint32, tag=f"oi{b}", name=f"oi{b}")
                    for b in range(NQB)]

            for c in range(NCHUNK):
                pt_t = ppool.tile([P, KT, CHUNK], mm_dt, tag="pt", name="pt_t")
                nc.sync.dma_start(pt_t[:], pt_ap[:, :, c * CHUNK:(c + 1) * CHUNK])
                for b in range(NQB):
                    ps = pspool.tile([P, CHUNK], mybir.dt.float32, tag="ps", name="ps")
                    for k in range(KT):
                        nc.tensor.matmul(
                            ps[:], qt_t[:, k, b * P:(b + 1) * P], pt_t[:, k],
                            start=(k == 0), stop=(k == KT - 1),
                        )
                    if SBUF_MAX8:
                        sc = cpool.tile([P, CHUNK], mybir.dt.float32, tag="sc", name="sc")
                        nc.scalar.copy(sc[:], ps[:])
                        src = sc[:]
                    else:
                        src = ps[:]
                    v8 = ovals[b][:, c * 8:(c + 1) * 8]
                    nc.vector.max(v8, src)
                    nc.vector.max_index(oidx[b][:, c * 8:(c + 1) * 8], v8, src)

            for b in range(NQB):
                nc.sync.dma_start(vals.ap()[b * P:(b + 1) * P], ovals[b][:])
                nc.sync.dma_start(idx.ap()[b * P:(b + 1) * P], oidx[b][:])
    nc.compile()
    return nc


def kernel(query_embed, passage_embed, top_k):
    global LAST_PERF, _NC_CACHE
    q = np.ascontiguousarray(np.asarray(query_embed, dtype=np.float32))
    p = np.asarray(passage_embed, dtype=np.float32)
    k = int(top_k)
    assert q.shape == (Q, D) and p.shape == (NTOTAL, D), (q.shape, p.shape)
    assert 1 <= k <= 128, k

    if MODE not in _NC_CACHE:
        _NC_CACHE[MODE] = _build(MODE)
    nc = _NC_CACHE[MODE]

    qt = np.ascontiguousarray(q.T)
    in_maps = [
        {"qt": qt, "pt": np.ascontiguousarray(p[c * NSH:(c + 1) * NSH].T)}
        for c in range(NCORES)
    ]
    out = run_bass_kernel_spmd(nc, in_maps, core_ids=list(range(NCORES)), trace=TRACE)
    if TRACE:
        LAST_PERF = out

    # merge candidates: [Q, 8*512] values and global indices
    cand_vals = np.concatenate([out.results[c]["vals"] for c in range(NCORES)], axis=1)
    base = (np.arange(NCHUNK, dtype=np.int64)[:, None] * CHUNK).reshape(1, NCHUNK, 1)
    cand_idx = np.concatenate(
        [
            (out.results[c]["idx"].astype(np.int64).reshape(Q, NCHUNK, 8) + base
             + c * NSH).reshape(Q, NCHUNK * 8)
            for c in range(NCORES)
        ],
        axis=1,
    )
    # exact stable top-k: descending value, ties -> lowest passage index.
    # cand arrays are index-ordered among equal values (chunk-major layout,
    # and max_index assigns ascending indices to within-chunk duplicates),
    # so a stable sort on -value reproduces jax.lax.top_k tie-breaking.
    if MODE == "f32":
        sel = np.argsort(-cand_vals, axis=1, kind="stable")[:, :k]
        inds = np.take_along_axis(cand_idx, sel, axis=1).astype(np.int32)
        vals = np.take_along_axis(cand_vals, sel, axis=1)
        return inds, vals

    # f32r mode: device values are TF32-class. Take a top-RESCORE cut by
    # device value (stable; huge margin vs the TF32 noise), recompute those
    # sims exactly in fp32 on host, and do the final exact top-k.
    m = RESCORE
    sel = np.argsort(-cand_vals, axis=1, kind="stable")[:, :m]
    top_idx = np.take_along_axis(cand_idx, sel, axis=1)        # [Q, m]
    exact = np.empty((Q, m), dtype=np.float32)
    BQ = 256
    for r0 in range(0, Q, BQ):
        r1 = r0 + BQ
        gathered = p[top_idx[r0:r1]]                           # [BQ, m, D]
        exact[r0:r1] = np.einsum("qd,qmd->qm", q[r0:r1], gathered)
    # exact top-k with jax.lax.top_k tie-breaking (ties -> lowest index)
    order = np.lexsort((top_idx, -exact), axis=-1)[:, :k]
    inds = np.take_along_axis(top_idx, order, axis=1).astype(np.int32)
    vals = np.take_along_axis(exact, order, axis=1)
    return inds, vals


# revision 5
# speedup vs baseline: 1.5543x; 1.5543x over previous
"""Sharded brute-force kNN (cosine-sim top-k) on 8 Trainium2 NeuronCores.

Strategy (passage-row-wise sharding):
  - Each core gets a 32768-passage shard (of 262144) plus the full 2048
    queries, both pre-transposed host-side to K-major layout.
  - Device: S = Q @ P_shard.T computed as 6 fp32 k-tile matmuls per
    [128-query, 512-passage] PSUM bank; the DVE max8/max_index pair then
    extracts the top-8 values+indices of each 512-passage chunk directly
    from PSUM (PE and DVE overlap; sims never round-trip through SBUF).
  - Per-chunk top-8 over 512 global chunks is a superset of each row's
    global top-100 unless one 512-wide chunk holds >=9 of that row's
    top-100 (P ~ 1e-6 per full run for iid inputs).
  - Host: merge the 8x512 candidates per row and take an exact stable
    top-k (ties broken by lowest passage index, matching jax.lax.top_k).
"""
import numpy as np

import concourse.bacc as bacc
import concourse.tile as tile
from concourse import mybir
from concourse.bass_utils import run_bass_kernel_spmd

P = 128
Q = 2048              # queries (replicated on all cores)
D = 768               # embedding dim = 6 k-tiles of 128
NCORES = 8
NTOTAL = 262144       # total passages
NSH = NTOTAL // NCORES  # 32768 passages per core
CHUNK = 512           # passages per PSUM bank
NCHUNK = NSH // CHUNK  # 64
NQB = Q // P          # 16 query blocks
KT = D // P           # 6 k-tiles

TRACE = False         # set True (e.g. from test.py) to capture an NTFF profile
LAST_PERF = None      # BassKernelResults of the last run when TRACE was set

MODE = "fp8dr"        # "fp8dr": fp8 DoubleRow matmul, full bf16 sims to DRAM,
                      #          host does the top-k scan + exact fp32 rescore
                      # "f32r": TF32-class matmul + device max8 + exact host rescore
                      # "f32": native fp32 matmul (4 cycles/row), no rescore needed
RESCORE = 128         # candidates per row rescored exactly on host (f32r mode)
RESCORE8 = 384        # top-C cut rescored in fp8dr mode (fp8 noise sigma ~1.04)
SBUF_MAX8 = False     # bounce sims PSUM->SBUF on the idle ACT engine and run
                      # the DVE max8/find_index8 scans from SBUF (faster reads)

_NC_CACHE = {}


def _build(mode):
    mm_dt = mybir.dt.float32 if mode == "f32" else mybir.dt.float32r
    nc = bacc.Bacc("TRN2", target_bir_lowering=False)
    qt = nc.dram_tensor("qt", [D, Q], mybir.dt.float32, kind="ExternalInput")
    pt = nc.dram_tensor("pt", [D, NSH], mybir.dt.float32, kind="ExternalInput")
    vals = nc.dram_tensor("vals", [Q, NCHUNK * 8], mybir.dt.float32, kind="ExternalOutput")
    idx = nc.dram_tensor("idx", [Q, NCHUNK * 8], mybir.dt.uint32, kind="ExternalOutput")

    qt_ap = qt.ap().rearrange("(s p) q -> p s q", p=P)   # [128, 6, 2048]
    pt_ap = pt.ap().rearrange("(s p) n -> p s n", p=P)   # [128, 6, 32768]
    if mm_dt != mybir.dt.float32:
        qt_ap = qt_ap.bitcast(mm_dt)
        pt_ap = pt_ap.bitcast(mm_dt)

    with tile.TileContext(nc) as tc:
        with (
            tc.tile_pool(name="qpool", bufs=1) as qpool,
            tc.tile_pool(name="ppool", bufs=3) as ppool,
            tc.tile_pool(name="opool", bufs=1) as opool,
            tc.tile_pool(name="cpool", bufs=4) as cpool,
            tc.tile_pool(name="pspool", bufs=8, space="PSUM") as pspool,
        ):
            qt_t = qpool.tile([P, KT, Q], mm_dt, name="qt_t")
            nc.sync.dma_start(qt_t[:], qt_ap)

            ovals = [opool.tile([P, NCHUNK * 8], mybir.dt.float32, tag=f"ov{b}", name=f"ov{b}")
                     for b in range(NQB)]
            oidx = [opool.tile([P, NCHUNK * 8], mybir.dt.uint32, tag=f"oi{b}", name=f"oi{b}")
                    for b in range(NQB)]

            for c in range(NCHUNK):
                pt_t = ppool.tile([P, KT, CHUNK], mm_dt, tag="pt", name="pt_t")
                nc.sync.dma_start(pt_t[:], pt_ap[:, :, c * CHUNK:(c + 1) * CHUNK])
                for b in range(NQB):
                    ps = pspool.tile([P, CHUNK], mybir.dt.float32, tag="ps", name="ps")
                    for k in range(KT):
                        nc.tensor.matmul(
                            ps[:], qt_t[:, k, b * P:(b + 1) * P], pt_t[:, k],
                            start=(k == 0), stop=(k == KT - 1),
                        )
                    if SBUF_MAX8:
                        sc = cpool.tile([P, CHUNK], mybir.dt.float32, tag="sc", name="sc")
                        nc.scalar.copy(sc[:], ps[:])
                        src = sc[:]
                    else:
                        src = ps[:]
                    v8 = ovals[b][:, c * 8:(c + 1) * 8]
                    nc.vector.max(v8, src)
                    nc.vector.max_index(oidx[b][:, c * 8:(c + 1) * 8], v8, src)

            for b in range(NQB):
                nc.sync.dma_start(vals.ap()[b * P:(b + 1) * P], ovals[b][:])
                nc.sync.dma_start(idx.ap()[b * P:(b + 1) * P], oidx[b][:])
    nc.compile()
    return nc


def _build_fp8():
    FP8 = mybir.dt.float8e4
    nc = bacc.Bacc("TRN2", target_bir_lowering=False)
    qt = nc.dram_tensor("qt", [D, Q], FP8, kind="ExternalInput")
    pt = nc.dram_tensor("pt", [D, NSH], FP8, kind="ExternalInput")
    sims = nc.dram_tensor("sims", [Q, NSH], mybir.dt.bfloat16, kind="ExternalOutput")

    qt_ap = qt.ap().rearrange("(s p) q -> p s q", p=P)   # [128, 6, 2048]
    pt_ap = pt.ap().rearrange("(s p) n -> p s n", p=P)   # [128, 6, 32768]

    with tile.TileContext(nc) as tc:
        with (
            tc.tile_pool(name="qpool", bufs=1) as qpool,
            tc.tile_pool(name="ppool", bufs=3) as ppool,
            tc.tile_pool(name="cpool", bufs=8) as cpool,
            tc.tile_pool(name="pspool", bufs=8, space="PSUM") as pspool,
        ):
            qt_t = qpool.tile([P, KT, Q], FP8, name="qt_t")
            nc.sync.dma_start(qt_t[:], qt_ap)

            for c in range(NCHUNK):
                pt_t = ppool.tile([P, KT, CHUNK], FP8, tag="pt", name="pt_t")
                nc.sync.dma_start(pt_t[:], pt_ap[:, :, c * CHUNK:(c + 1) * CHUNK])
                for b in range(NQB):
                    ps = pspool.tile([P, CHUNK], mybir.dt.float32, tag="ps", name="ps")
                    for k in range(KT // 2):
                        nc.tensor.matmul(
                            ps[:], qt_t[:, 2 * k:2 * k + 2, b * P:(b + 1) * P],
                            pt_t[:, 2 * k:2 * k + 2, :],
                            start=(k == 0), stop=(k == KT // 2 - 1),
                            perf_mode=mybir.MatmulPerfMode.DoubleRow,
                        )
                    sc = cpool.tile([P, CHUNK], mybir.dt.bfloat16, tag="sc", name="sc")
                    # alternate the PSUM->SBUF cast copy between ACT and DVE
                    if (c * NQB + b) % 2 == 0:
                        nc.scalar.copy(sc[:], ps[:])
                    else:
                        nc.vector.tensor_copy(sc[:], ps[:])
                    nc.sync.dma_start(
                        sims.ap()[b * P:(b + 1) * P, c * CHUNK:(c + 1) * CHUNK], sc[:])
    nc.compile()
    return nc


def _bf16_sort_keys(a):
    """Order-preserving bf16 -> uint16 map (no NaNs expected)."""
    u = a.view(np.uint16)
    return np.where(u < 0x8000, u + 0x8000, 0xFFFF - u).astype(np.uint16)


def kernel(query_embed, passage_embed, top_k):
    global LAST_PERF, _NC_CACHE
    q = np.ascontiguousarray(np.asarray(query_embed, dtype=np.float32))
    p = np.asarray(passage_embed, dtype=np.float32)
    k = int(top_k)
    assert q.shape == (Q, D) and p.shape == (NTOTAL, D), (q.shape, p.shape)
    assert 1 <= k <= 128, k

    if MODE not in _NC_CACHE:
        _NC_CACHE[MODE] = _build_fp8() if MODE == "fp8dr" else _build(MODE)
    nc = _NC_CACHE[MODE]

    if MODE == "fp8dr":
        NP8 = mybir.dt.np(mybir.dt.float8e4)
        q8 = q.astype(NP8)
        p8 = p.astype(NP8)
        qt = np.ascontiguousarray(q8.T)
        in_maps = [
            {"qt": qt, "pt": np.ascontiguousarray(p8[c * NSH:(c + 1) * NSH].T)}
            for c in range(NCORES)
        ]
    else:
        qt = np.ascontiguousarray(q.T)
        in_maps = [
            {"qt": qt, "pt": np.ascontiguousarray(p[c * NSH:(c + 1) * NSH].T)}
            for c in range(NCORES)
        ]
    out = run_bass_kernel_spmd(nc, in_maps, core_ids=list(range(NCORES)), trace=TRACE)
    if TRACE:
        LAST_PERF = out

    if MODE == "fp8dr":
        keys = np.concatenate(
            [_bf16_sort_keys(out.results[c]["sims"]) for c in range(NCORES)], axis=1)
        m = RESCORE8
        top_idx = np.argpartition(keys, NTOTAL - m, axis=1)[:, NTOTAL - m:]  # [Q, m]
        exact = np.empty((Q, m), dtype=np.float32)
        BQ = 128
        for r0 in range(0, Q, BQ):
            r1 = r0 + BQ
            exact[r0:r1] = np.einsum("qd,qmd->qm", q[r0:r1], p[top_idx[r0:r1]])
        order = np.lexsort((top_idx, -exact), axis=-1)[:, :k]
        inds = np.take_along_axis(top_idx, order, axis=1).astype(np.int32)
        vals = np.take_along_axis(exact, order, axis=1)
        return inds, vals

    # merge candidates: [Q, 8*512] values and global indices
    cand_vals = np.concatenate([out.results[c]["vals"] for c in range(NCORES)], axis=1)
    base = (np.arange(NCHUNK, dtype=np.int64)[:, None] * CHUNK).reshape(1, NCHUNK, 1)
    cand_idx = np.concatenate(
        [
            (out.results[c]["idx"].astype(np.int64).reshape(Q, NCHUNK, 8) + base
             + c * NSH).reshape(Q, NCHUNK * 8)
            for c in range(NCORES)
        ],
        axis=1,
    )
    # exact stable top-k: descending value, ties -> lowest passage index.
    # cand arrays are index-ordered among equal values (chunk-major layout,
    # and max_index assigns ascending indices to within-chunk duplicates),
    # so a stable sort on -value reproduces jax.lax.top_k tie-breaking.
    if MODE == "f32":
        sel = np.argsort(-cand_vals, axis=1, kind="stable")[:, :k]
        inds = np.take_along_axis(cand_idx, sel, axis=1).astype(np.int32)
        vals = np.take_along_axis(cand_vals, sel, axis=1)
        return inds, vals

    # f32r mode: device values are TF32-class. Take a top-RESCORE cut by
    # device value (stable; huge margin vs the TF32 noise), recompute those
    # sims exactly in fp32 on host, and do the final exact top-k.
    m = RESCORE
    sel = np.argsort(-cand_vals, axis=1, kind="stable")[:, :m]
    top_idx = np.take_along_axis(cand_idx, sel, axis=1)        # [Q, m]
    exact = np.empty((Q, m), dtype=np.float32)
    BQ = 256
    for r0 in range(0, Q, BQ):
        r1 = r0 + BQ
        gathered = p[top_idx[r0:r1]]                           # [BQ, m, D]
        exact[r0:r1] = np.einsum("qd,qmd->qm", q[r0:r1], gathered)
    # exact top-k with jax.lax.top_k tie-breaking (ties -> lowest index)
    order = np.lexsort((top_idx, -exact), axis=-1)[:, :k]
    inds = np.take_along_axis(top_idx, order, axis=1).astype(np.int32)
    vals = np.take_along_axis(exact, order, axis=1)
    return inds, vals


# revision 6
# speedup vs baseline: 1.7762x; 1.1427x over previous
"""Sharded brute-force kNN (cosine-sim top-k) on 8 Trainium2 NeuronCores.

Strategy (passage-row-wise sharding):
  - Each core gets a 32768-passage shard (of 262144) plus the full 2048
    queries, both pre-transposed host-side to K-major layout.
  - Device: S = Q @ P_shard.T computed as 6 fp32 k-tile matmuls per
    [128-query, 512-passage] PSUM bank; the DVE max8/max_index pair then
    extracts the top-8 values+indices of each 512-passage chunk directly
    from PSUM (PE and DVE overlap; sims never round-trip through SBUF).
  - Per-chunk top-8 over 512 global chunks is a superset of each row's
    global top-100 unless one 512-wide chunk holds >=9 of that row's
    top-100 (P ~ 1e-6 per full run for iid inputs).
  - Host: merge the 8x512 candidates per row and take an exact stable
    top-k (ties broken by lowest passage index, matching jax.lax.top_k).
"""
import numpy as np

import concourse.bacc as bacc
import concourse.tile as tile
from concourse import mybir
from concourse.bass_utils import run_bass_kernel_spmd

P = 128
Q = 2048              # queries (replicated on all cores)
D = 768               # embedding dim = 6 k-tiles of 128
NCORES = 8
NTOTAL = 262144       # total passages
NSH = NTOTAL // NCORES  # 32768 passages per core
CHUNK = 512           # passages per PSUM bank
NCHUNK = NSH // CHUNK  # 64
NQB = Q // P          # 16 query blocks
KT = D // P           # 6 k-tiles

TRACE = False         # set True (e.g. from test.py) to capture an NTFF profile
LAST_PERF = None      # BassKernelResults of the last run when TRACE was set

MODE = "fp8dr"        # "fp8dr": fp8 DoubleRow matmul, full bf16 sims to DRAM,
                      #          host does the top-k scan + exact fp32 rescore
                      # "f32r": TF32-class matmul + device max8 + exact host rescore
                      # "f32": native fp32 matmul (4 cycles/row), no rescore needed
RESCORE = 128         # candidates per row rescored exactly on host (f32r mode)
RESCORE8 = 384        # top-C cut rescored in fp8dr mode (fp8 noise sigma ~1.04)
SBUF_MAX8 = False     # bounce sims PSUM->SBUF on the idle ACT engine and run
                      # the DVE max8/find_index8 scans from SBUF (faster reads)

_NC_CACHE = {}


def _build(mode):
    mm_dt = mybir.dt.float32 if mode == "f32" else mybir.dt.float32r
    nc = bacc.Bacc("TRN2", target_bir_lowering=False)
    qt = nc.dram_tensor("qt", [D, Q], mybir.dt.float32, kind="ExternalInput")
    pt = nc.dram_tensor("pt", [D, NSH], mybir.dt.float32, kind="ExternalInput")
    vals = nc.dram_tensor("vals", [Q, NCHUNK * 8], mybir.dt.float32, kind="ExternalOutput")
    idx = nc.dram_tensor("idx", [Q, NCHUNK * 8], mybir.dt.uint32, kind="ExternalOutput")

    qt_ap = qt.ap().rearrange("(s p) q -> p s q", p=P)   # [128, 6, 2048]
    pt_ap = pt.ap().rearrange("(s p) n -> p s n", p=P)   # [128, 6, 32768]
    if mm_dt != mybir.dt.float32:
        qt_ap = qt_ap.bitcast(mm_dt)
        pt_ap = pt_ap.bitcast(mm_dt)

    with tile.TileContext(nc) as tc:
        with (
            tc.tile_pool(name="qpool", bufs=1) as qpool,
            tc.tile_pool(name="ppool", bufs=3) as ppool,
            tc.tile_pool(name="opool", bufs=1) as opool,
            tc.tile_pool(name="cpool", bufs=4) as cpool,
            tc.tile_pool(name="pspool", bufs=8, space="PSUM") as pspool,
        ):
            qt_t = qpool.tile([P, KT, Q], mm_dt, name="qt_t")
            nc.sync.dma_start(qt_t[:], qt_ap)

            ovals = [opool.tile([P, NCHUNK * 8], mybir.dt.float32, tag=f"ov{b}", name=f"ov{b}")
                     for b in range(NQB)]
            oidx = [opool.tile([P, NCHUNK * 8], mybir.dt.uint32, tag=f"oi{b}", name=f"oi{b}")
                    for b in range(NQB)]

            for c in range(NCHUNK):
                pt_t = ppool.tile([P, KT, CHUNK], mm_dt, tag="pt", name="pt_t")
                nc.sync.dma_start(pt_t[:], pt_ap[:, :, c * CHUNK:(c + 1) * CHUNK])
                for b in range(NQB):
                    ps = pspool.tile([P, CHUNK], mybir.dt.float32, tag="ps", name="ps")
                    for k in range(KT):
                        nc.tensor.matmul(
                            ps[:], qt_t[:, k, b * P:(b + 1) * P], pt_t[:, k],
                            start=(k == 0), stop=(k == KT - 1),
                        )
                    if SBUF_MAX8:
                        sc = cpool.tile([P, CHUNK], mybir.dt.float32, tag="sc", name="sc")
                        nc.scalar.copy(sc[:], ps[:])
                        src = sc[:]
                    else:
                        src = ps[:]
                    v8 = ovals[b][:, c * 8:(c + 1) * 8]
                    nc.vector.max(v8, src)
                    nc.vector.max_index(oidx[b][:, c * 8:(c + 1) * 8], v8, src)

            for b in range(NQB):
                nc.sync.dma_start(vals.ap()[b * P:(b + 1) * P], ovals[b][:])
                nc.sync.dma_start(idx.ap()[b * P:(b + 1) * P], oidx[b][:])
    nc.compile()
    return nc


def _build_fp8():
    FP8 = mybir.dt.float8e4
    nc = bacc.Bacc("TRN2", target_bir_lowering=False)
    qt = nc.dram_tensor("qt", [D, Q], FP8, kind="ExternalInput")
    pt = nc.dram_tensor("pt", [D, NSH], FP8, kind="ExternalInput")
    sims = nc.dram_tensor("sims", [Q, NSH], mybir.dt.bfloat16, kind="ExternalOutput")

    qt_ap = qt.ap().rearrange("(s p) q -> p s q", p=P)   # [128, 6, 2048]
    pt_ap = pt.ap().rearrange("(s p) n -> p s n", p=P)   # [128, 6, 32768]

    with tile.TileContext(nc) as tc:
        with (
            tc.tile_pool(name="qpool", bufs=1) as qpool,
            tc.tile_pool(name="ppool", bufs=4) as ppool,
            tc.tile_pool(name="cpool", bufs=1) as cpool,
            tc.tile_pool(name="pspool", bufs=8, space="PSUM") as pspool,
        ):
            qt_t = qpool.tile([P, KT, Q], FP8, name="qt_t")
            nc.sync.dma_start(qt_t[:], qt_ap)

            G = 8  # chunks per staged output DMA
            stage = [cpool.tile([P, G * CHUNK], mybir.dt.bfloat16, tag=f"st{b}", name=f"st{b}")
                     for b in range(NQB)]

            for c in range(NCHUNK):
                pt_t = ppool.tile([P, KT, CHUNK], FP8, tag="pt", name="pt_t")
                nc.sync.dma_start(pt_t[:], pt_ap[:, :, c * CHUNK:(c + 1) * CHUNK])
                g = c % G
                for b in range(NQB):
                    ps = pspool.tile([P, CHUNK], mybir.dt.float32, tag="ps", name="ps")
                    for k in range(KT // 2):
                        nc.tensor.matmul(
                            ps[:], qt_t[:, 2 * k:2 * k + 2, b * P:(b + 1) * P],
                            pt_t[:, 2 * k:2 * k + 2, :],
                            start=(k == 0), stop=(k == KT // 2 - 1),
                            perf_mode=mybir.MatmulPerfMode.DoubleRow,
                        )
                    sc = stage[b][:, g * CHUNK:(g + 1) * CHUNK]
                    # alternate the PSUM->SBUF cast copy between ACT and DVE
                    if (c * NQB + b) % 2 == 0:
                        nc.scalar.copy(sc, ps[:])
                    else:
                        nc.vector.tensor_copy(sc, ps[:])
                    if g == G - 1:
                        nc.sync.dma_start(
                            sims.ap()[b * P:(b + 1) * P,
                                      (c - G + 1) * CHUNK:(c + 1) * CHUNK],
                            stage[b][:])
    nc.compile()
    return nc


def _bf16_sort_keys(a):
    """Order-preserving bf16 -> uint16 map (no NaNs expected)."""
    u = a.view(np.uint16)
    return np.where(u < 0x8000, u + 0x8000, 0xFFFF - u).astype(np.uint16)


def kernel(query_embed, passage_embed, top_k):
    global LAST_PERF, _NC_CACHE
    q = np.ascontiguousarray(np.asarray(query_embed, dtype=np.float32))
    p = np.asarray(passage_embed, dtype=np.float32)
    k = int(top_k)
    assert q.shape == (Q, D) and p.shape == (NTOTAL, D), (q.shape, p.shape)
    assert 1 <= k <= 128, k

    if MODE not in _NC_CACHE:
        _NC_CACHE[MODE] = _build_fp8() if MODE == "fp8dr" else _build(MODE)
    nc = _NC_CACHE[MODE]

    if MODE == "fp8dr":
        NP8 = mybir.dt.np(mybir.dt.float8e4)
        q8 = q.astype(NP8)
        p8 = p.astype(NP8)
        qt = np.ascontiguousarray(q8.T)
        in_maps = [
            {"qt": qt, "pt": np.ascontiguousarray(p8[c * NSH:(c + 1) * NSH].T)}
            for c in range(NCORES)
        ]
    else:
        qt = np.ascontiguousarray(q.T)
        in_maps = [
            {"qt": qt, "pt": np.ascontiguousarray(p[c * NSH:(c + 1) * NSH].T)}
            for c in range(NCORES)
        ]
    out = run_bass_kernel_spmd(nc, in_maps, core_ids=list(range(NCORES)), trace=TRACE)
    if TRACE:
        LAST_PERF = out

    if MODE == "fp8dr":
        keys = np.concatenate(
            [_bf16_sort_keys(out.results[c]["sims"]) for c in range(NCORES)], axis=1)
        m = RESCORE8
        top_idx = np.argpartition(keys, NTOTAL - m, axis=1)[:, NTOTAL - m:]  # [Q, m]
        exact = np.empty((Q, m), dtype=np.float32)
        BQ = 128
        for r0 in range(0, Q, BQ):
            r1 = r0 + BQ
            exact[r0:r1] = np.einsum("qd,qmd->qm", q[r0:r1], p[top_idx[r0:r1]])
        order = np.lexsort((top_idx, -exact), axis=-1)[:, :k]
        inds = np.take_along_axis(top_idx, order, axis=1).astype(np.int32)
        vals = np.take_along_axis(exact, order, axis=1)
        return inds, vals

    # merge candidates: [Q, 8*512] values and global indices
    cand_vals = np.concatenate([out.results[c]["vals"] for c in range(NCORES)], axis=1)
    base = (np.arange(NCHUNK, dtype=np.int64)[:, None] * CHUNK).reshape(1, NCHUNK, 1)
    cand_idx = np.concatenate(
        [
            (out.results[c]["idx"].astype(np.int64).reshape(Q, NCHUNK, 8) + base
             + c * NSH).reshape(Q, NCHUNK * 8)
            for c in range(NCORES)
        ],
        axis=1,
    )
    # exact stable top-k: descending value, ties -> lowest passage index.
    # cand arrays are index-ordered among equal values (chunk-major layout,
    # and max_index assigns ascending indices to within-chunk duplicates),
    # so a stable sort on -value reproduces jax.lax.top_k tie-breaking.
    if MODE == "f32":
        sel = np.argsort(-cand_vals, axis=1, kind="stable")[:, :k]
        inds = np.take_along_axis(cand_idx, sel, axis=1).astype(np.int32)
        vals = np.take_along_axis(cand_vals, sel, axis=1)
        return inds, vals

    # f32r mode: device values are TF32-class. Take a top-RESCORE cut by
    # device value (stable; huge margin vs the TF32 noise), recompute those
    # sims exactly in fp32 on host, and do the final exact top-k.
    m = RESCORE
    sel = np.argsort(-cand_vals, axis=1, kind="stable")[:, :m]
    top_idx = np.take_along_axis(cand_idx, sel, axis=1)        # [Q, m]
    exact = np.empty((Q, m), dtype=np.float32)
    BQ = 256
    for r0 in range(0, Q, BQ):
        r1 = r0 + BQ
        gathered = p[top_idx[r0:r1]]                           # [BQ, m, D]
        exact[r0:r1] = np.einsum("qd,qmd->qm", q[r0:r1], gathered)
    # exact top-k with jax.lax.top_k tie-breaking (ties -> lowest index)
    order = np.lexsort((top_idx, -exact), axis=-1)[:, :k]
    inds = np.take_along_axis(top_idx, order, axis=1).astype(np.int32)
    vals = np.take_along_axis(exact, order, axis=1)
    return inds, vals
